# revision 44
# baseline (speedup 1.0000x reference)
"""Causal self-attention (B=2, T=4096, C=768, H=12) on 8 trn2 NeuronCores.

Sharding: core c -> batch b = c//4, head group g = c%4 (3 heads per core).
Each core computes QKV projection for its 3 heads, causal attention, and a
partial output projection (its heads' contribution to y @ w_proj, transposed).
Host sums the 4 partials per batch and adds b_proj.

On-device layout (per core):
  - x^T (C, T) bf16 is the only activation input.
  - Q^T/K^T (64, T) per head come straight out of W-stationary matmuls.
    3 QK m-tiles: [q0;q1], [k0;k1], [q2;k2] (no padding).
  - Attention runs in the S^T orientation: S^T[k, q] = K^T.T @ Q^T tiles, so
    softmax exp runs on ScalarE with no max pass (logits are O(1) for this
    problem) and no P transpose is ever needed.
  - PV runs FLIPPED: Yf[q, hd] = P^T.T @ V' with lhsT = 128-col P^T slices
    (FWL-eligible) -> M=128 output partitions (full array) instead of 66,
    and the softmax denominator l (from a ones-column in V') lands
    per-PARTITION, so normalization is reciprocal_approx_fast + one
    broadcast multiply -- no PE ones-broadcast machinery.
  - Normalized Yf (bf16) is transposed back to [hd, q] by PE transposes
    (h0/h1 packed into one [128,128] transpose) for the output projection.
  - Output projection packs h0+h1 into K=128 matmuls (+ h2 at K=64),
    emitting partial^T (C, T) bf16.
"""

import os
import sys

import numpy as np

for _p in ("/opt/trn_rl_repo", "/root/.axon_site/_ro/trn_rl_repo"):
    if os.path.isdir(_p) and _p not in sys.path:
        sys.path.insert(0, _p)

import ml_dtypes

import concourse.bacc as bacc
import concourse.bass as bass
import concourse.mybir as mybir
import concourse.tile as tile
from concourse.bass_utils import run_bass_kernel_spmd

B, T, C = 2, 4096, 768
H, HD = 12, 64
NCORES = 8
HPC = 3  # heads per core
P = 128
NBLK = T // 512  # 8 q-blocks of 512
NKT = T // 128  # 32 k-tiles of 128
KC = C // 128  # 6 contraction chunks
VW = HD + 1  # V' width (ones col for the softmax denominator)

F32 = mybir.dt.float32
BF16 = mybir.dt.bfloat16
FP8 = mybir.dt.float8e4
BF16_NP = ml_dtypes.bfloat16
FP8_NP = ml_dtypes.float8_e4m3fn
AF = mybir.ActivationFunctionType
WSC = 32.0  # fp8 weight pre-scale (keeps w*0.02 out of e4m3 denormals)
KC8 = 3  # fp8 DoubleRow contraction chunks of 256

_CACHE = {}
_DEBUG = False


def _build_nc():
    nc = bacc.Bacc("TRN2", target_bir_lowering=False, debug=False)

    xt_d = nc.dram_tensor("xt", [C, T], BF16, kind="ExternalInput")
    xt8_d = nc.dram_tensor("xt8", [C, T], FP8, kind="ExternalInput")
    wqk_d = nc.dram_tensor("wqk", [C, 384], FP8, kind="ExternalInput")
    wv_d = nc.dram_tensor("wv", [C, HPC * HD], BF16, kind="ExternalInput")
    wp_d = nc.dram_tensor("wp", [HPC * HD, C], BF16, kind="ExternalInput")
    scale_d = nc.dram_tensor("scale_qk", [P, 3], F32, kind="ExternalInput")
    bias_d = nc.dram_tensor("bias_qk", [P, 3], F32, kind="ExternalInput")
    bv_d = nc.dram_tensor("bv", [P, HPC * HD], F32, kind="ExternalInput")
    mask_d = nc.dram_tensor("mask", [P, P], BF16, kind="ExternalInput")
    ident_d = nc.dram_tensor("ident", [P, P], BF16, kind="ExternalInput")
    out_d = nc.dram_tensor("outT", [C, T], BF16, kind="ExternalOutput")

    with tile.TileContext(nc) as tc:
        with (
            tc.tile_pool(name="store", bufs=1) as store,
            tc.tile_pool(name="consts", bufs=1) as consts,
            tc.tile_pool(name="pt_pool", bufs=5) as pt_pool,
            tc.tile_pool(name="ynf_pool", bufs=2) as ynf_pool,
            tc.tile_pool(name="r_pool", bufs=4) as r_pool,
            tc.tile_pool(name="osb_pool", bufs=3) as osb_pool,
            tc.tile_pool(name="s_psum", bufs=2, space="PSUM") as s_psum,
            tc.tile_pool(name="y_psum", bufs=2, space="PSUM") as y_psum,
            tc.tile_pool(name="m_psum", bufs=2, space="PSUM") as m_psum,
        ):
            # ---- persistent SBUF storage ----
            # Q/K projections run in fp8 DoubleRow ([P, chunk, Ko=2, free],
            # contraction 256 per chunk); the V projection stays bf16 -- fp8
            # V feeds the output directly on peaked (early-token) rows and
            # blows the error budget, while Q/K noise is softmax-laundered.
            XT = store.tile([P, KC, T], BF16)
            XT8 = store.tile([P, KC8, 2, T], FP8)
            WQK = store.tile([P, KC8, 2, 384], FP8)
            WV = store.tile([P, KC, HPC * HD], BF16)
            WP01 = store.tile([P, C], BF16)  # w_proj rows: h0 @0-63, h1 @64-127
            WP2 = store.tile([HD, C], BF16)
            QT01 = store.tile([P, T], BF16)  # Q^T h0 @0-63, h1 @64-127
            KT01 = store.tile([P, T], BF16)
            QK2 = store.tile([P, T], BF16)  # Q^T h2 @0-63, K^T h2 @64-127
            KT2 = store.tile([HD, T], BF16)  # K^T h2 re-homed to base 0 (DMA)
            VN = store.tile([P, NKT, HPC, VW], BF16)  # V' with ones col
            YN01 = store.tile([P, T], BF16)  # normalized Y^T: h0 | h1 stacked
            YN2 = store.tile([HD, T], BF16)

            zl = consts.tile([1, P], BF16)  # K=1 zero operands: one matmul
            zr = consts.tile([1, 512], BF16)  # zeroes a whole PSUM bank
            scale_qk = consts.tile([P, 3], F32)
            bias_qk = consts.tile([P, 3], F32)
            bvb = consts.tile([P, HPC * HD], F32)  # host-prebroadcast bias
            mask = consts.tile([P, P], BF16)
            ident = consts.tile([P, P], BF16)

            # ---- input DMAs (weights/consts first so the prologue's QKV
            # matmuls can start as soon as x^T chunk 0 lands) ----
            nc.sync.dma_start(
                WQK[:], wqk_d.rearrange("(k o p) c -> p k o c", o=2, p=P)
            )
            nc.sync.dma_start(WV[:], wv_d.rearrange("(k p) c -> p k c", p=P))
            nc.sync.dma_start(scale_qk[:], scale_d[:])
            nc.sync.dma_start(bias_qk[:], bias_d[:])
            nc.sync.dma_start(bvb[:], bv_d[:])
            nc.sync.dma_start(mask[:], mask_d[:])
            nc.sync.dma_start(ident[:], ident_d[:])
            nc.sync.dma_start(WP01[:], wp_d[0:P, :])
            nc.sync.dma_start(WP2[:], wp_d[P : P + HD, :])
            xt8_view = xt8_d.rearrange("(k o p) t -> p k o t", o=2, p=P)
            xt_view = xt_d.rearrange("(k p) t -> p k t", p=P)
            for n in range(NBLK):
                nc.sync.dma_start(
                    XT8[:, :, :, n * 512 : (n + 1) * 512],
                    xt8_view[:, :, :, n * 512 : (n + 1) * 512],
                )
                nc.sync.dma_start(
                    XT[:, :, n * 512 : (n + 1) * 512],
                    xt_view[:, :, n * 512 : (n + 1) * 512],
                )

            nc.any.memset(VN[:, :, :, HD : HD + 1], 1.0)
            nc.any.memset(zl[:], 0.0)
            nc.any.memset(zr[:], 0.0)

            # ---- work-group builders ----
            # M-tiles of wqk cols: 0=[qh0;qh1] 1=[kh0;kh1] 2=[qh2;kh2]
            def qkv_group(m, n):
                ps = m_psum.tile([P, 512], F32, tag="misc")
                for k in range(KC8):
                    nc.tensor.matmul(
                        ps[:],
                        WQK[:, k, :, m * P : (m + 1) * P],
                        XT8[:, k, :, n * 512 : (n + 1) * 512],
                        start=(k == 0),
                        stop=(k == KC8 - 1),
                        perf_mode=mybir.MatmulPerfMode.DoubleRow,
                    )
                dst = (QT01, KT01, QK2)[m]
                nc.vector.tensor_scalar(
                    dst[:, n * 512 : (n + 1) * 512],
                    ps[:],
                    scale_qk[:, m : m + 1],
                    bias_qk[:, m : m + 1],
                    op0=mybir.AluOpType.mult,
                    op1=mybir.AluOpType.add,
                )
                if m == 2:
                    # matmul needs lhsT/rhs at the same base partition; move
                    # K^T h2 from partitions 64-127 down to base 0 via DMA
                    nc.sync.dma_start(
                        KT2[:, n * 512 : (n + 1) * 512],
                        QK2[64:128, n * 512 : (n + 1) * 512],
                    )

            def v_group(mt):
                vp = m_psum.tile([P, HPC * HD], F32, tag="misc")
                for k in range(KC):
                    nc.tensor.matmul(
                        vp[:],
                        XT[:, k, mt * P : (mt + 1) * P],
                        WV[:, k, :],
                        start=(k == 0),
                        stop=(k == KC - 1),
                    )
                # bias folded into the PSUM->SBUF eviction
                nc.vector.tensor_add(
                    VN[:, mt, :, 0:HD],
                    vp[:].rearrange("p (h d) -> p h d", h=HPC),
                    bvb[:].rearrange("p (h d) -> p h d", h=HPC),
                )

            def proj_group(m, n):
                ops = m_psum.tile([P, 512], F32, tag="misc")
                nc.tensor.matmul(
                    ops[:],
                    WP01[:, m * P : (m + 1) * P],
                    YN01[:, n * 512 : (n + 1) * 512],
                    start=True,
                    stop=False,
                )
                nc.tensor.matmul(
                    ops[:],
                    WP2[:, m * P : (m + 1) * P],
                    YN2[:, n * 512 : (n + 1) * 512],
                    start=False,
                    stop=True,
                )
                osb = osb_pool.tile([P, 512], BF16)
                nc.vector.tensor_copy(osb[:], ops[:])
                nc.sync.dma_start(
                    out_d[m * P : (m + 1) * P, n * 512 : (n + 1) * 512],
                    osb[:],
                )

            # ---- filler queue: PE work drip-fed into the attention phase so
            # the tensor engine never micro-idles (keeps HAM at K=8/8) ----
            from collections import deque

            filler_q = deque()
            chunk_done = [0]  # chunks fully emitted (prologue: 0)

            def pop_filler(k):
                for _ in range(k):
                    if not filler_q:
                        return
                    n_final, fn = filler_q.popleft()
                    fn()
                    if n_final is not None:
                        chunk_done[0] = max(chunk_done[0], n_final)

            def drain_through_chunk(n):
                while filler_q and chunk_done[0] < n:
                    pop_filler(1)

            # deferred post-block work (PE transposes / DVE evictions),
            # popped a couple of chunks into the following head-blocks so
            # neither engine stalls at a block boundary
            deferred = deque()

            def pop_deferred():
                if deferred:
                    deferred.popleft()()

            # ---- attention head-block (S^T + flipped PV, LAG-pipelined) ----
            heads = (
                (QT01, KT01, 0, 0),
                (QT01, KT01, HD, HD),
                (QK2, KT2, 0, 0),
            )
            LAG = 2  # PV trails S by LAG exp-chunks

            def emit_pv(h, i, yps, ent):
                pt, off0, off1, j0, j1 = ent
                for j, base, off in ((j0, 0, off0), (j1, 512, off1)):
                    for qt in range(4):
                        if qt * P < off:
                            continue
                        # start=False always: the bank was cleared by one
                        # zero-matmul (hardware first_mm clears has_written
                        # for the WHOLE bank, so per-qt starts would wipe
                        # sibling regions)
                        nc.tensor.matmul(
                            yps[:, qt, 0:VW],
                            pt[:, base + qt * P : base + (qt + 1) * P],
                            VN[:, j, h, :],
                            start=False,
                            stop=(j == 4 * i + 3 and qt == 3),
                            skip_group_check=True,
                        )

            def attn_block(h, i, ynf2, ynf1):
                QTt, KTt, qoff, koff = heads[h]
                # qtile slot padded to 128 f32 so matmul outputs stay
                # 512B-aligned within the PSUM bank
                yps = y_psum.tile([P, 4, P], F32)
                # one K=1 zero-matmul clears the whole bank (hardware
                # first_mm clears has_written bank-wide, so per-qt starts
                # would wipe sibling regions); PVs accumulate with
                # start=False on top
                nc.tensor.matmul(
                    yps[:].rearrange("p a b -> p (a b)"),
                    zl[:],
                    zr[:],
                    start=True,
                    stop=False,
                )
                pending = []
                for c in range(2 * i + 2):
                    j0, j1 = 2 * c, 2 * c + 1
                    off0 = max(0, j0 - 4 * i) * P
                    off1 = max(0, j1 - 4 * i) * P
                    sps = s_psum.tile([P, 1024], F32)
                    nc.tensor.matmul(
                        sps[:, off0:512],
                        KTt[koff : koff + HD, j0 * P : (j0 + 1) * P],
                        QTt[qoff : qoff + HD, i * 512 + off0 : (i + 1) * 512],
                        start=True,
                        stop=True,
                    )
                    nc.tensor.matmul(
                        sps[:, 512 + off1 : 1024],
                        KTt[koff : koff + HD, j1 * P : (j1 + 1) * P],
                        QTt[qoff : qoff + HD, i * 512 + off1 : (i + 1) * 512],
                        start=True,
                        stop=True,
                    )
                    pt = pt_pool.tile([P, 1024], BF16)
                    if off1 > off0:
                        # diagonal chunk: skip the unwritten PSUM gap
                        nc.scalar.activation(
                            pt[:, off0:512], sps[:, off0:512], AF.Exp
                        )
                        nc.scalar.activation(
                            pt[:, 512 + off1 :], sps[:, 512 + off1 :], AF.Exp
                        )
                    else:
                        nc.scalar.activation(pt[:, off0:], sps[:, off0:], AF.Exp)
                    # causal mask: zero the upper-k triangle of diagonal units
                    # (multiplicative, post-exp -> off the S->exp critical path)
                    if j0 >= 4 * i:
                        nc.vector.tensor_mul(
                            pt[:, off0 : off0 + P], pt[:, off0 : off0 + P], mask[:]
                        )
                    if j1 >= 4 * i:
                        nc.vector.tensor_mul(
                            pt[:, 512 + off1 : 512 + off1 + P],
                            pt[:, 512 + off1 : 512 + off1 + P],
                            mask[:],
                        )
                    pending.append((pt, off0, off1, j0, j1))
                    if len(pending) > LAG:
                        emit_pv(h, i, yps, pending.pop(0))
                    if c == 2 or c == 4:
                        pop_deferred()
                    if c % 6 == 5:
                        pop_filler(1)
                # Filler BEFORE the pending-PV drain: the lagged PVs wait on
                # the last exps, which have no S-work left to hide behind.
                pop_filler(1 + (i + 1) // 2)
                while pending:
                    emit_pv(h, i, yps, pending.pop(0))
                # normalize: r = 1/l (l sits per-partition in the ones col),
                # then one broadcast multiply into the bf16 staging tile
                r = r_pool.tile([P, 4, 1], F32)
                nc.vector.reciprocal_approx_fast(r[:], yps[:, :, HD : HD + 1])
                dst = (
                    ynf2[:, :, 0:HD],
                    ynf2[:, :, HD : 2 * HD],
                    ynf1[:, :, :],
                )[h]
                nc.vector.tensor_mul(
                    dst, yps[:, :, 0:HD], r[:].broadcast_to([P, 4, HD])
                )
                if _DEBUG and i == 0:
                    nc.vector.tensor_copy(dbg_yps[h][:], yps[:])

            def make_block_epilogue(i, ynf2, ynf1):
                # PE transposes back to [hd, q] + DVE evictions; proj fillers
                # are appended once their YN inputs have been emitted
                def t_pair():
                    tp = m_psum.tile([P, 4, P], BF16, tag="misc")
                    for qt in range(4):
                        nc.tensor.transpose(tp[:, qt, :], ynf2[:, qt, :], ident[:])
                    t_pair.tp = tp

                def e_pair():
                    nc.vector.tensor_copy(
                        YN01[:, i * 512 : (i + 1) * 512],
                        t_pair.tp[:].rearrange("p a b -> p (a b)"),
                    )

                def t_single():
                    tp = m_psum.tile([HD, 4, P], BF16, tag="misc")
                    for qt in range(4):
                        nc.tensor.transpose(tp[:, qt, :], ynf1[:, qt, :], ident[:])
                    t_single.tp = tp

                def e_single():
                    nc.vector.tensor_copy(
                        YN2[:, i * 512 : (i + 1) * 512],
                        t_single.tp[:].rearrange("p a b -> p (a b)"),
                    )
                    for m in range(KC):
                        filler_q.append((None, lambda m=m, n=i: proj_group(m, n)))

                deferred.extend([t_pair, e_pair, t_single, e_single])

            if _DEBUG:
                dbg_yps = [
                    store.tile([P, 4, P], F32, name=f"dbg_yps_t{_h}")
                    for _h in range(3)
                ]
                dbg_ynf2 = store.tile([P, 4, P], BF16)
                dbg_ynf1 = store.tile([P, 4, HD], BF16)

            # ---- prologue: QKV + V for token chunk 0 (dense PE warmup) ----
            for m in range(3):
                qkv_group(m, 0)
            for mt in range(4):
                v_group(mt)

            # remaining chunks become filler work; chunk n is complete once
            # its last group (the v tile 4n+3) has been emitted
            for n in range(1, NBLK):
                for m in range(3):
                    filler_q.append((None, lambda m=m, n=n: qkv_group(m, n)))
                for j in range(4):
                    t = 4 * n + j
                    filler_q.append((n if j == 3 else None, lambda t=t: v_group(t)))

            # ---- main pipeline ----
            for i in range(NBLK):
                drain_through_chunk(i)
                ynf2 = ynf_pool.tile([P, 4, P], BF16, tag="pair")
                ynf1 = ynf_pool.tile([P, 4, HD], BF16, tag="single")
                for h in range(HPC):
                    attn_block(h, i, ynf2, ynf1)
                if _DEBUG and i == 0:
                    nc.vector.tensor_copy(dbg_ynf2[:], ynf2[:])
                    nc.vector.tensor_copy(dbg_ynf1[:], ynf1[:])
                pop_filler(2)
                make_block_epilogue(i, ynf2, ynf1)

            while deferred:
                pop_deferred()
            while filler_q:
                pop_filler(1)

            if _DEBUG:
                for name, t in (
                    ("dbg_qt01", QT01),
                    ("dbg_kt01", KT01),
                    ("dbg_qk2", QK2),
                    ("dbg_kt2", KT2),
                    ("dbg_yn01", YN01),
                    ("dbg_yn2", YN2),
                ):
                    d = nc.dram_tensor(
                        name, list(t.shape), BF16, kind="ExternalOutput"
                    )
                    nc.sync.dma_start(d[:], t[:])
                dv = nc.dram_tensor(
                    "dbg_vn", [P, NKT, HPC, VW], BF16, kind="ExternalOutput"
                )
                nc.sync.dma_start(dv[:], VN[:])
                for hh in range(3):
                    dy = nc.dram_tensor(
                        f"dbg_yps{hh}", [P, 4, P], F32, kind="ExternalOutput"
                    )
                    nc.sync.dma_start(dy[:], dbg_yps[hh][:])
                d2 = nc.dram_tensor(
                    "dbg_ynf2", [P, 4, P], BF16, kind="ExternalOutput"
                )
                nc.sync.dma_start(d2[:], dbg_ynf2[:])
                d1 = nc.dram_tensor(
                    "dbg_ynf1", [P, 4, HD], BF16, kind="ExternalOutput"
                )
                nc.sync.dma_start(d1[:], dbg_ynf1[:])

    nc.compile()
    return nc


def _per_core_inputs(c, x, w_attn, b_attn, xt_cache):
    b, g = divmod(c, 4)
    hs = [HPC * g + j for j in range(HPC)]

    if b not in xt_cache:
        xtb = np.ascontiguousarray(x[b].T)
        xt_cache[b] = (xtb.astype(BF16_NP), xtb.astype(FP8_NP))
    xt, xt8 = xt_cache[b]

    qc = lambda h: w_attn[:, h * HD : (h + 1) * HD]
    kc = lambda h: w_attn[:, C + h * HD : C + (h + 1) * HD]
    vc = lambda h: w_attn[:, 2 * C + h * HD : 2 * C + (h + 1) * HD]
    wqk = (
        WSC
        * np.concatenate(
            [qc(hs[0]), qc(hs[1]), kc(hs[0]), kc(hs[1]), qc(hs[2]), kc(hs[2])],
            axis=1,
        )
    ).astype(FP8_NP)
    wv = np.concatenate([vc(h) for h in hs], axis=1).astype(BF16_NP)

    bq = lambda h: b_attn[h * HD : (h + 1) * HD]
    bk = lambda h: b_attn[C + h * HD : C + (h + 1) * HD]
    sc = 1.0 / np.sqrt(np.float32(HD))
    bias_qk = np.stack(
        [
            np.concatenate([bq(hs[0]), bq(hs[1])]) * sc,
            np.concatenate([bk(hs[0]), bk(hs[1])]),
            np.concatenate([bq(hs[2]) * sc, bk(hs[2])]),
        ],
        axis=1,
    ).astype(np.float32)
    scale_qk = (
        np.stack(
            [
                np.full(P, sc),
                np.ones(P),
                np.concatenate([np.full(HD, sc), np.ones(HD)]),
            ],
            axis=1,
        )
        / WSC  # undo the fp8 weight pre-scale
    ).astype(np.float32)
    bv = np.broadcast_to(
        np.concatenate(
            [b_attn[2 * C + h * HD : 2 * C + (h + 1) * HD] for h in hs]
        ).astype(np.float32)[None, :],
        (P, HPC * HD),
    ).copy()

    # multiplicative causal mask: keep k<=q (partition p = local k, col c = q)
    mask = (np.arange(P)[:, None] <= np.arange(P)[None, :]).astype(BF16_NP)
    ident = np.eye(P, dtype=BF16_NP)

    return {
        "xt": xt,
        "xt8": xt8,
        "wqk": wqk,
        "wv": wv,
        "wp": None,  # filled by caller (shared per group)
        "scale_qk": scale_qk,
        "bias_qk": bias_qk,
        "bv": bv,
        "mask": mask,
        "ident": ident,
    }


def build_in_maps(x, w_attn, b_attn, w_proj):
    x = np.asarray(x, np.float32)
    w_attn = np.asarray(w_attn, np.float32)
    b_attn = np.asarray(b_attn, np.float32)
    w_proj = np.asarray(w_proj, np.float32)

    xt_cache = {}
    in_maps = []
    for c in range(NCORES):
        m = _per_core_inputs(c, x, w_attn, b_attn, xt_cache)
        g = c % 4
        hs = [HPC * g + j for j in range(HPC)]
        m["wp"] = np.concatenate(
            [w_proj[h * HD : (h + 1) * HD, :] for h in hs], axis=0
        ).astype(BF16_NP)
        in_maps.append(m)
    return in_maps


def kernel(x, w_attn, b_attn, w_proj, b_proj, _return_raw=False):
    x = np.asarray(x, np.float32)
    b_proj = np.asarray(b_proj, np.float32)

    if "nc" not in _CACHE:
        _CACHE["nc"] = _build_nc()
    nc = _CACHE["nc"]

    in_maps = build_in_maps(x, w_attn, b_attn, w_proj)
    res = run_bass_kernel_spmd(nc, in_maps, list(range(NCORES)))
    outs = [r["outT"] for r in res.results]

    full = np.empty((B, T, C), np.float32)
    for b in range(B):
        acc = outs[4 * b].astype(np.float32)
        for g in range(1, 4):
            acc += outs[4 * b + g].astype(np.float32)
        full[b] = acc.T
    full += b_proj[None, None, :]
    if _return_raw:
        return full, res
    return full


# revision 58
# speedup vs baseline: 1.2247x; 1.2247x over previous
"""Causal self-attention (B=2, T=4096, C=768, H=12) on 8 trn2 NeuronCores.

Sharding: core c -> batch b = c//4, head group g = c%4 (3 heads per core).
Each core computes QKV projection for its 3 heads, causal attention, and a
partial output projection (its heads' contribution to y @ w_proj, transposed).
Host sums the 4 partials per batch and adds b_proj.

On-device layout (per core):
  - x^T (C, T) arrives twice: fp8 e4m3 for the Q/K projections (DoubleRow
    matmuls, contraction 256/chunk) and bf16 for the V projection (fp8 V
    feeds the output directly on peaked early-token rows and blows the
    error budget; Q/K noise is softmax-laundered).
  - Q^T/K^T (64, T) per head come from W-stationary matmuls; 3 m-tiles
    [q0;q1], [k0;k1], [q2;k2] (no padding; K^T h2 re-homed to base
    partition 0 by an SBUF->SBUF DMA).
  - Attention runs in the S^T orientation: S^T[k, q] = K^T.T @ Q^T tiles, so
    softmax exp runs on ScalarE with no max pass (logits are O(1) for this
    problem) and no P transpose is ever needed.
  - PV is V'-stationary (M=65: 64 y rows + a ones-column row accumulating
    the softmax denominator l), N=512 streams of P^T -- fully pipelined
    213ns matmuls.
  - Normalization: reciprocal_approx_fast on the l row, bf16 K=1
    ones-broadcast matmul, one DVE multiply -> normalized Y^T bf16.
    h1's Y^T is DMA'd from partitions 0-63 into YN01[64:128] so the
    output projection can contract h0+h1 in K=128 matmuls (+ h2 at K=64),
    emitting partial^T (C, T) bf16.
"""

import os
import sys

import numpy as np

for _p in ("/opt/trn_rl_repo", "/root/.axon_site/_ro/trn_rl_repo"):
    if os.path.isdir(_p) and _p not in sys.path:
        sys.path.insert(0, _p)

import ml_dtypes

import concourse.bacc as bacc
import concourse.bass as bass
import concourse.mybir as mybir
import concourse.tile as tile
from concourse.bass_utils import run_bass_kernel_spmd

B, T, C = 2, 4096, 768
H, HD = 12, 64
NCORES = 8
HPC = 3  # heads per core
P = 128
NBLK = T // 512  # 8 q-blocks of 512
NKT = T // 128  # 32 k-tiles of 128
KC = C // 128  # 6 bf16 contraction chunks
KC8 = 3  # fp8 DoubleRow contraction chunks of 256
VW = HD + 2  # V' width (ones col -> l row; +1 pad to keep 4B slot alignment)

F32 = mybir.dt.float32
BF16 = mybir.dt.bfloat16
FP8 = mybir.dt.float8e4
BF16_NP = ml_dtypes.bfloat16
FP8_NP = ml_dtypes.float8_e4m3fn
AF = mybir.ActivationFunctionType
WSC = 32.0  # fp8 weight pre-scale (keeps w*0.02 out of e4m3 denormals)

_CACHE = {}
_DEBUG = False


def _build_nc():
    nc = bacc.Bacc("TRN2", target_bir_lowering=False, debug=False)

    xt_d = nc.dram_tensor("xt", [C, T], BF16, kind="ExternalInput")
    xt8_d = nc.dram_tensor("xt8", [C, T], FP8, kind="ExternalInput")
    wqk_d = nc.dram_tensor("wqk", [C, 384], FP8, kind="ExternalInput")
    wv_d = nc.dram_tensor("wv", [C, HPC * HD], BF16, kind="ExternalInput")
    wp_d = nc.dram_tensor("wp", [HPC * HD, C], BF16, kind="ExternalInput")
    scale_d = nc.dram_tensor("scale_qk", [P, 3], F32, kind="ExternalInput")
    bias_d = nc.dram_tensor("bias_qk", [P, 3], F32, kind="ExternalInput")
    bv_d = nc.dram_tensor("bv", [P, HPC * HD], F32, kind="ExternalInput")
    mask_d = nc.dram_tensor("mask", [P, P], BF16, kind="ExternalInput")
    out_d = nc.dram_tensor("outT", [C, T], BF16, kind="ExternalOutput")

    with tile.TileContext(nc) as tc:
        with (
            tc.tile_pool(name="store", bufs=1) as store,
            tc.tile_pool(name="consts", bufs=1) as consts,
            tc.tile_pool(name="pt_pool", bufs=5) as pt_pool,
            tc.tile_pool(name="r_pool", bufs=3) as r_pool,
            tc.tile_pool(name="stg_pool", bufs=4) as stg_pool,
            tc.tile_pool(name="osb_pool", bufs=3) as osb_pool,
            tc.tile_pool(name="s_psum", bufs=2, space="PSUM") as s_psum,
            tc.tile_pool(name="y_psum", bufs=2, space="PSUM") as y_psum,
            tc.tile_pool(name="m_psum", bufs=2, space="PSUM") as m_psum,
        ):
            # ---- persistent SBUF storage ----
            XT = store.tile([P, KC, T], BF16)  # x^T bf16 (V projection)
            XT8 = store.tile([P, KC8, 2, T], FP8)  # x^T fp8 (Q/K DoubleRow)
            WQK = store.tile([P, KC8, 2, 384], FP8)
            WV = store.tile([P, KC, HPC * HD], BF16)
            WP01 = store.tile([P, C], BF16)  # w_proj rows: h0 @0-63, h1 @64-127
            WP2 = store.tile([HD, C], BF16)
            QT01 = store.tile([P, T], BF16)  # Q^T h0 @0-63, h1 @64-127
            KT01 = store.tile([P, T], BF16)
            QK2 = store.tile([P, T], BF16)  # Q^T h2 @0-63, K^T h2 @64-127
            KT2 = store.tile([HD, T], BF16)  # K^T h2 re-homed to base 0 (DMA)
            VN = store.tile([P, NKT, HPC, VW], BF16)  # V' with ones col
            YN01 = store.tile([P, T], BF16)  # normalized Y^T: h0 | h1 stacked
            YN2 = store.tile([HD, T], BF16)

            scale_qk = consts.tile([P, 3], F32)
            bias_qk = consts.tile([P, 3], F32)
            bvb = consts.tile([P, HPC * HD], F32)  # host-prebroadcast bias
            mask = consts.tile([P, P], BF16)
            onesrow = consts.tile([HD + 1, HD], BF16)  # row 64 used as lhsT

            # ---- input DMAs (weights/consts first so the prologue's QKV
            # matmuls can start as soon as x^T chunk 0 lands) ----
            nc.sync.dma_start(
                WQK[:], wqk_d.rearrange("(k o p) c -> p k o c", o=2, p=P)
            )
            nc.sync.dma_start(WV[:], wv_d.rearrange("(k p) c -> p k c", p=P))
            nc.sync.dma_start(scale_qk[:], scale_d[:])
            nc.sync.dma_start(bias_qk[:], bias_d[:])
            nc.sync.dma_start(bvb[:], bv_d[:])
            nc.sync.dma_start(mask[:], mask_d[:])
            nc.sync.dma_start(WP01[:], wp_d[0:P, :])
            nc.sync.dma_start(WP2[:], wp_d[P : P + HD, :])
            xt8_view = xt8_d.rearrange("(k o p) t -> p k o t", o=2, p=P)
            xt_view = xt_d.rearrange("(k p) t -> p k t", p=P)
            for n in range(NBLK):
                nc.sync.dma_start(
                    XT8[:, :, :, n * 512 : (n + 1) * 512],
                    xt8_view[:, :, :, n * 512 : (n + 1) * 512],
                )
                nc.sync.dma_start(
                    XT[:, :, n * 512 : (n + 1) * 512],
                    xt_view[:, :, n * 512 : (n + 1) * 512],
                )

            nc.any.memset(VN[:, :, :, HD : HD + 2], 1.0)
            nc.any.memset(onesrow[:], 1.0)

            # ---- work-group builders ----
            # M-tiles of wqk cols: 0=[qh0;qh1] 1=[kh0;kh1] 2=[qh2;kh2]
            def qkv_group(m, n):
                ps = m_psum.tile([P, 512], F32, tag="misc")
                for k in range(KC8):
                    nc.tensor.matmul(
                        ps[:],
                        WQK[:, k, :, m * P : (m + 1) * P],
                        XT8[:, k, :, n * 512 : (n + 1) * 512],
                        start=(k == 0),
                        stop=(k == KC8 - 1),
                        perf_mode=mybir.MatmulPerfMode.DoubleRow,
                    )
                dst = (QT01, KT01, QK2)[m]
                nc.vector.tensor_scalar(
                    dst[:, n * 512 : (n + 1) * 512],
                    ps[:],
                    scale_qk[:, m : m + 1],
                    bias_qk[:, m : m + 1],
                    op0=mybir.AluOpType.mult,
                    op1=mybir.AluOpType.add,
                )
                if m == 2:
                    # matmul needs lhsT/rhs at the same base partition; move
                    # K^T h2 from partitions 64-127 down to base 0 via DMA
                    nc.sync.dma_start(
                        KT2[:, n * 512 : (n + 1) * 512],
                        QK2[64:128, n * 512 : (n + 1) * 512],
                    )

            def v_group(mt):
                vp = m_psum.tile([P, HPC * HD], F32, tag="misc")
                for k in range(KC):
                    nc.tensor.matmul(
                        vp[:],
                        XT[:, k, mt * P : (mt + 1) * P],
                        WV[:, k, :],
                        start=(k == 0),
                        stop=(k == KC - 1),
                    )
                # bias folded into the PSUM->SBUF eviction
                nc.vector.tensor_add(
                    VN[:, mt, :, 0:HD],
                    vp[:].rearrange("p (h d) -> p h d", h=HPC),
                    bvb[:].rearrange("p (h d) -> p h d", h=HPC),
                )

            def proj_group(m, n):
                ops = m_psum.tile([P, 512], F32, tag="misc")
                nc.tensor.matmul(
                    ops[:],
                    WP01[:, m * P : (m + 1) * P],
                    YN01[:, n * 512 : (n + 1) * 512],
                    start=True,
                    stop=False,
                )
                nc.tensor.matmul(
                    ops[:],
                    WP2[:, m * P : (m + 1) * P],
                    YN2[:, n * 512 : (n + 1) * 512],
                    start=False,
                    stop=True,
                )
                osb = osb_pool.tile([P, 512], BF16)
                nc.vector.tensor_copy(osb[:], ops[:])
                nc.sync.dma_start(
                    out_d[m * P : (m + 1) * P, n * 512 : (n + 1) * 512],
                    osb[:],
                )

            # ---- filler queue: PE work drip-fed into the attention phase so
            # the tensor engine never micro-idles (keeps HAM at K=8/8) ----
            from collections import deque

            filler_q = deque()
            chunk_done = [0]  # chunks fully emitted (prologue: 0)

            def pop_filler(k):
                for _ in range(k):
                    if not filler_q:
                        return
                    n_final, fn = filler_q.popleft()
                    fn()
                    if n_final is not None:
                        chunk_done[0] = max(chunk_done[0], n_final)

            def drain_through_chunk(n):
                while filler_q and chunk_done[0] < n:
                    pop_filler(1)

            # deferred normalize closures (PE ones-broadcast + DVE multiply),
            # popped a couple of chunks into the following head-blocks so the
            # PE never waits on the DVE reciprocal at a block boundary
            deferred = deque()

            def pop_deferred():
                if deferred:
                    deferred.popleft()()

            # ---- attention head-block (S^T orientation, LAG-pipelined) ----
            heads = (
                (QT01, KT01, 0, 0),
                (QT01, KT01, HD, HD),
                (QK2, KT2, 0, 0),
            )
            LAG = 2  # PV trails S by LAG exp-chunks

            def emit_pv(h, i, yps, ent):
                pt, off0, off1, j0, j1 = ent
                nc.tensor.matmul(
                    yps[:, off0:],
                    VN[:, j0, h, :],
                    pt[:, off0:512],
                    start=(j0 == 0),
                    stop=False,
                )
                nc.tensor.matmul(
                    yps[:, off1:],
                    VN[:, j1, h, :],
                    pt[:, 512 + off1 : 1024],
                    start=False,
                    stop=(j1 == 4 * i + 3),
                )

            def attn_block(h, i):
                QTt, KTt, qoff, koff = heads[h]
                yps = y_psum.tile([VW, 512], F32)  # y^T rows 0-63, l row 64
                pending = []
                for c in range(2 * i + 2):
                    j0, j1 = 2 * c, 2 * c + 1
                    off0 = max(0, j0 - 4 * i) * P
                    off1 = max(0, j1 - 4 * i) * P
                    sps = s_psum.tile([P, 1024], F32)
                    nc.tensor.matmul(
                        sps[:, off0:512],
                        KTt[koff : koff + HD, j0 * P : (j0 + 1) * P],
                        QTt[qoff : qoff + HD, i * 512 + off0 : (i + 1) * 512],
                        start=True,
                        stop=True,
                    )
                    nc.tensor.matmul(
                        sps[:, 512 + off1 : 1024],
                        KTt[koff : koff + HD, j1 * P : (j1 + 1) * P],
                        QTt[qoff : qoff + HD, i * 512 + off1 : (i + 1) * 512],
                        start=True,
                        stop=True,
                    )
                    pt = pt_pool.tile([P, 1024], BF16)
                    if off1 > off0:
                        # diagonal chunk: skip the unwritten PSUM gap
                        nc.scalar.activation(
                            pt[:, off0:512], sps[:, off0:512], AF.Exp
                        )
                        nc.scalar.activation(
                            pt[:, 512 + off1 :], sps[:, 512 + off1 :], AF.Exp
                        )
                    else:
                        nc.scalar.activation(pt[:, off0:], sps[:, off0:], AF.Exp)
                    # causal mask: zero the upper-k triangle of diagonal units
                    # (multiplicative, post-exp -> off the S->exp critical path)
                    if j0 >= 4 * i:
                        nc.vector.tensor_mul(
                            pt[:, off0 : off0 + P], pt[:, off0 : off0 + P], mask[:]
                        )
                    if j1 >= 4 * i:
                        nc.vector.tensor_mul(
                            pt[:, 512 + off1 : 512 + off1 + P],
                            pt[:, 512 + off1 : 512 + off1 + P],
                            mask[:],
                        )
                    pending.append((pt, off0, off1, j0, j1))
                    if len(pending) > LAG:
                        emit_pv(h, i, yps, pending.pop(0))
                    if c == 2 or c == 4:
                        pop_deferred()
                    if c % 6 == 5:
                        pop_filler(1)
                # Filler BEFORE the pending-PV drain: the lagged PVs wait on
                # the last exps, which have no S-work left to hide behind.
                pop_filler(1 + (i + 1) // 2)
                while pending:
                    emit_pv(h, i, yps, pending.pop(0))
                # normalize, stage 1 (DVE, inline): r = 1/l on the l row;
                # stage unnormalized y^T to SBUF (DVE reads only one PSUM
                # operand, so the later multiply needs y in SBUF)
                rf = r_pool.tile([HD + 1, 512], F32, tag="rf")
                rb = r_pool.tile([HD + 1, 512], BF16, tag="rb")
                yt = stg_pool.tile([HD, 512], BF16)
                nc.vector.reciprocal_approx_fast(rf[0 : HD + 1, :], yps[0 : HD + 1, :])
                nc.vector.tensor_copy(rb[HD : HD + 1, :], rf[HD : HD + 1, :])
                nc.vector.tensor_copy(yt[:], yps[0:HD, :])

                # stage 2 (deferred): K=1 ones-broadcast of r on the PE, then
                # one DVE multiply -> normalized bf16 Y^T
                def bcast_mul(h=h, i=i, yt=yt, rb=rb):
                    rps = m_psum.tile([HD, 512], F32, tag="misc")
                    nc.tensor.matmul(
                        rps[:],
                        onesrow[HD : HD + 1, :],
                        rb[HD : HD + 1, :],
                        start=True,
                        stop=True,
                    )
                    if h == 0:
                        nc.vector.tensor_mul(
                            YN01[0:HD, i * 512 : (i + 1) * 512], yt[:], rps[:]
                        )
                    elif h == 1:
                        # DVE can't shift partitions; stage at base 0 and let
                        # a DMA re-home it to YN01 rows 64-127
                        stg = stg_pool.tile([HD, 512], BF16, tag="h1")
                        nc.vector.tensor_mul(stg[:], yt[:], rps[:])
                        nc.sync.dma_start(
                            YN01[HD:P, i * 512 : (i + 1) * 512], stg[:]
                        )
                    else:
                        nc.vector.tensor_mul(
                            YN2[:, i * 512 : (i + 1) * 512], yt[:], rps[:]
                        )
                        for m in range(KC):
                            filler_q.append(
                                (None, lambda m=m, n=i: proj_group(m, n))
                            )

                deferred.append(bcast_mul)

            # ---- prologue: QKV + V for token chunk 0 (dense PE warmup) ----
            for m in range(3):
                qkv_group(m, 0)
            for mt in range(4):
                v_group(mt)

            # remaining chunks become filler work; chunk n is complete once
            # its last group (the v tile 4n+3) has been emitted
            for n in range(1, NBLK):
                for m in range(3):
                    filler_q.append((None, lambda m=m, n=n: qkv_group(m, n)))
                for j in range(4):
                    t = 4 * n + j
                    filler_q.append((n if j == 3 else None, lambda t=t: v_group(t)))

            # ---- main pipeline ----
            for i in range(NBLK):
                drain_through_chunk(i)
                for h in range(HPC):
                    attn_block(h, i)
                pop_filler(2)

            while deferred:
                pop_deferred()
            while filler_q:
                pop_filler(1)

            if _DEBUG:
                for name, t in (
                    ("dbg_qt01", QT01),
                    ("dbg_kt01", KT01),
                    ("dbg_qk2", QK2),
                    ("dbg_kt2", KT2),
                    ("dbg_yn01", YN01),
                    ("dbg_yn2", YN2),
                ):
                    d = nc.dram_tensor(
                        name, list(t.shape), BF16, kind="ExternalOutput"
                    )
                    nc.sync.dma_start(d[:], t[:])
                dv = nc.dram_tensor(
                    "dbg_vn", [P, NKT, HPC, VW], BF16, kind="ExternalOutput"
                )
                nc.sync.dma_start(dv[:], VN[:])

    nc.compile()
    return nc


def _per_core_inputs(c, x, w_attn, b_attn, xt_cache):
    b, g = divmod(c, 4)
    hs = [HPC * g + j for j in range(HPC)]

    if b not in xt_cache:
        xtb = np.ascontiguousarray(x[b].T)
        xt_cache[b] = (xtb.astype(BF16_NP), xtb.astype(FP8_NP))
    xt, xt8 = xt_cache[b]

    qc = lambda h: w_attn[:, h * HD : (h + 1) * HD]
    kc = lambda h: w_attn[:, C + h * HD : C + (h + 1) * HD]
    vc = lambda h: w_attn[:, 2 * C + h * HD : 2 * C + (h + 1) * HD]
    wqk = (
        WSC
        * np.concatenate(
            [qc(hs[0]), qc(hs[1]), kc(hs[0]), kc(hs[1]), qc(hs[2]), kc(hs[2])],
            axis=1,
        )
    ).astype(FP8_NP)
    wv = np.concatenate([vc(h) for h in hs], axis=1).astype(BF16_NP)

    bq = lambda h: b_attn[h * HD : (h + 1) * HD]
    bk = lambda h: b_attn[C + h * HD : C + (h + 1) * HD]
    sc = 1.0 / np.sqrt(np.float32(HD))
    bias_qk = np.stack(
        [
            np.concatenate([bq(hs[0]), bq(hs[1])]) * sc,
            np.concatenate([bk(hs[0]), bk(hs[1])]),
            np.concatenate([bq(hs[2]) * sc, bk(hs[2])]),
        ],
        axis=1,
    ).astype(np.float32)
    scale_qk = (
        np.stack(
            [
                np.full(P, sc),
                np.ones(P),
                np.concatenate([np.full(HD, sc), np.ones(HD)]),
            ],
            axis=1,
        )
        / WSC  # undo the fp8 weight pre-scale
    ).astype(np.float32)
    bv = np.broadcast_to(
        np.concatenate(
            [b_attn[2 * C + h * HD : 2 * C + (h + 1) * HD] for h in hs]
        ).astype(np.float32)[None, :],
        (P, HPC * HD),
    ).copy()

    # multiplicative causal mask: keep k<=q (partition p = local k, col c = q)
    mask = (np.arange(P)[:, None] <= np.arange(P)[None, :]).astype(BF16_NP)

    return {
        "xt": xt,
        "xt8": xt8,
        "wqk": wqk,
        "wv": wv,
        "wp": None,  # filled by caller (shared per group)
        "scale_qk": scale_qk,
        "bias_qk": bias_qk,
        "bv": bv,
        "mask": mask,
    }


def build_in_maps(x, w_attn, b_attn, w_proj):
    x = np.asarray(x, np.float32)
    w_attn = np.asarray(w_attn, np.float32)
    b_attn = np.asarray(b_attn, np.float32)
    w_proj = np.asarray(w_proj, np.float32)

    xt_cache = {}
    in_maps = []
    for c in range(NCORES):
        m = _per_core_inputs(c, x, w_attn, b_attn, xt_cache)
        g = c % 4
        hs = [HPC * g + j for j in range(HPC)]
        m["wp"] = np.concatenate(
            [w_proj[h * HD : (h + 1) * HD, :] for h in hs], axis=0
        ).astype(BF16_NP)
        in_maps.append(m)
    return in_maps


def kernel(x, w_attn, b_attn, w_proj, b_proj, _return_raw=False):
    x = np.asarray(x, np.float32)
    b_proj = np.asarray(b_proj, np.float32)

    if "nc" not in _CACHE:
        _CACHE["nc"] = _build_nc()
    nc = _CACHE["nc"]

    in_maps = build_in_maps(x, w_attn, b_attn, w_proj)
    res = run_bass_kernel_spmd(nc, in_maps, list(range(NCORES)))
    outs = [r["outT"] for r in res.results]

    full = np.empty((B, T, C), np.float32)
    for b in range(B):
        acc = outs[4 * b].astype(np.float32)
        for g in range(1, 4):
            acc += outs[4 * b + g].astype(np.float32)
        full[b] = acc.T
    full += b_proj[None, None, :]
    if _return_raw:
        return full, res
    return full
